# revision 33
# baseline (speedup 1.0000x reference)
"""AffineCoupling TRN2 kernel (v27, ~283us vs ~344us for the v8 baseline).

Computes, for z [4_000_000, 16] fp32:
    zl = z[:, :8]; zr = z[:, 8:]
    log_s = MLP_logs(zl); b = MLP_b(zl)        (5 layers, LeakyReLU(0.01) between)
    out = concat([zl, exp(log_s) * zr + b], axis=1)

Strategy:
 - All feature-major transforms run on the DVE StreamTranspose (32x32
   block transpose) instead of Pool-gather + PE-identity-matmuls +
   PSUM->SBUF copies.  Layout: nat[p, s*1024 + g*16 + f] holds row
   r = r0 + s*8192 + p*64 + g.  Choosing MLP partition q = 32B + gl2*8 + f
   (group G = 4B + gl2) keeps every row inside its 32-partition block, so
   ONE StreamTranspose with in-AP [[64,32],[16,4],[1,8]] produces
   x0[q, 32j+b] = zbf[row(j,b,G), f] directly (j = 16s + jg,
   row = s*8192 + (32B+b)*64 + 4*jg + gl2).  Same trick with offset 8
   gives zr feature-major, and the inverse (fp32) writes yr back into
   nat in place.  This removes ~3.5us/macro of Pool gather, ~3.1us of
   PE transpose+ldweights, and ~2.4us of PSUM->SBUF copies.
 - No bf16 cast anywhere: the MLP matmuls run in float32r (1 cycle/row
   at >=256 moving cols, ~11-bit effective mantissa), so x0/zr/h tiles
   stay fp32 and the StreamTransposes read nat fp32 directly.
 - LeakyReLU: 6 instances on ACT (1-op Prelu) + 2 on DVE via a custom
   1-op DVE op LEAKY_BIAS_ANT: out = max(t, t*0.01), t = in + bias.
 - Tail: tmp = e * zr_fm on Pool (fp32);
   yr_fm = (hp5b + bias_b5) + tmp via one DVE STT (fp32);
   yr StreamTranspose fp32 back into nat; one nat-sized out-DMA.
 - PE runs ONLY the 20 MLP matmuls (10240 cycles/macro).
 - Steady state is DVE-bound at ~7.9us/macro (DVE ~98% busy: 3 STs +
   fp32r-rounding copy + STT + 2 custom leakys).  Input DMA is
   prefetched 4 macros deep (NAT_BUFS=10) — shallower prefetch stalls
   the head StreamTransposes and costs ~25us.  Emission order within an
   iteration is load-bearing: the L5/exp pair is split so exp lands
   mid-iteration (its product feeds next iteration's Pool tmp, whose
   consumer STT is the first DVE op with an unmet dep), and ST-zr runs
   before the STT so DVE has ready work while Pool computes tmp.
"""
import os
import sys

sys.path.insert(0, "/opt/trn_rl_repo")
if "/root/.axon_site/_ro/trn_rl_repo" not in sys.path:
    sys.path.append("/root/.axon_site/_ro/trn_rl_repo")

import numpy as np

import concourse.bacc as bacc
import concourse.bass as bass
import concourse.tile as tile
from concourse import mybir
from concourse.bass import _add_dep_helper
from concourse.bass_utils import run_bass_kernel_spmd

FP = mybir.dt.float32
FR = mybir.dt.float32r
BF = mybir.dt.bfloat16

N_CORES = 8
BATCH = 4_000_000
ROWS_PER_MACRO = 16_384            # [128, 2048] nat tile
MACROS = 31
R = ROWS_PER_MACRO * MACROS        # 507,904 rows per core
PAD_ROWS = ROWS_PER_MACRO          # guard band: writes never touch tensor tail
NAT_BUFS = 10

STEP = 498_688
STARTS = [c * STEP for c in range(N_CORES - 1)] + [BATCH - R]

C_BIAS = 0
C_TOTAL = 10
W_TOTAL = 10 * 128                 # 10 block-diag lhsT blocks

LAST_RESULTS = None

ALPHA = 0.01

# engine assignment for the 8 LeakyReLU instances ("act" = 1-op Prelu on
# the Activation engine, "dve" = 1-op custom LEAKY_BIAS_ANT on Vector)
LEAKY_ENGINE = {
    (0, "s"): "act", (0, "b"): "act",
    (1, "s"): "act", (1, "b"): "act",
    (2, "s"): "act", (2, "b"): "act",
    (3, "s"): "dve", (3, "b"): "dve",
}

_LEAKY_OP = None


def _register_leaky():
    """Register the 1-op DVE LeakyReLU-with-bias; idempotent."""
    global _LEAKY_OP
    if _LEAKY_OP is not None:
        return _LEAKY_OP
    from concourse import dve_ops
    from concourse.dve_spec import Spec, Src0, C0, C2, maxx, lower, _has_src1
    from concourse.dve_uop import DveOpSpec

    name = "LEAKY_BIAS_ANT"
    for op in dve_ops.OPS:
        if op.name == name:
            _LEAKY_OP = op
            return op
    t = Src0 + C0
    spec = Spec(
        body=maxx(t, t * C2),
        reference=lambda in0, s0, s1, imm2: np.maximum(
            in0.astype(np.float32) + s0, (in0.astype(np.float32) + s0) * imm2
        ),
    )
    row = dve_ops._CUSTOM_DVE_ROW_BASE + len(dve_ops.OPS)
    assert row < 0x20
    shas = {}
    for ver in ("v3", "v4"):
        s = DveOpSpec(name=name, opcode=row, uops=lower(spec, ver=ver),
                      rd1_en=_has_src1(spec))
        shas[ver] = s.sha(ver)
    op = dve_ops.DveOp(name, spec, subdim=False, uops_sha=shas)
    dve_ops.OPS.append(op)
    dve_ops.CUSTOM_DVE_SPECS[name] = spec
    dve_ops._SUB_OPCODE_FOR_NAME[name] = row
    _LEAKY_OP = op
    return op


def _build_consts(ws_logs, bs_logs, ws_b, bs_b):
    import ml_dtypes

    ws_logs = [np.asarray(w, np.float32) for w in ws_logs]
    bs_logs = [np.asarray(b, np.float32) for b in bs_logs]
    ws_b = [np.asarray(w, np.float32) for w in ws_b]
    bs_b = [np.asarray(b, np.float32) for b in bs_b]

    consts = np.zeros((128, C_TOTAL), np.float32)
    # bias columns: index = low 3 bits of partition q -> tile(bias, 16)
    for k in range(4):
        consts[:, C_BIAS + k] = np.tile(bs_logs[k], 16)
        consts[:, C_BIAS + 4 + k] = np.tile(bs_b[k], 16)
    consts[:, C_BIAS + 8] = np.tile(bs_logs[4], 16)   # exp bias
    consts[:, C_BIAS + 9] = np.tile(bs_b[4], 16)      # b5 bias

    wmat = np.zeros((128, W_TOTAL), np.float32)
    for k in range(5):
        for bi, ws in ((0, ws_logs[k]), (1, ws_b[k])):
            blk = k * 2 + bi
            lhsT = np.zeros((128, 128), np.float32)
            for t in range(16):          # t = q-group; base = t*8 in q-order
                base = t * 8
                lhsT[base:base + 8, base:base + 8] = ws.T
            wmat[:, blk * 128:(blk + 1) * 128] = lhsT
    return consts, wmat


def _ap(t, offset, dims):
    """AP over tile t keeping its partition dim, explicit free dims
    [[step, count], ...] and an element offset into the free space."""
    return bass.AP(tensor=t.tensor, offset=t.offset + offset, ap=[t.ap[0]] + dims)


ST_DIMS = [[64, 32], [16, 4], [1, 8]]   # feature-major <-> nat StreamTranspose AP


def _build_nc():
    leaky_op = _register_leaky()

    nc = bacc.Bacc()
    z_d = nc.declare_dram_parameter("z", [R + PAD_ROWS, 16], FP, isOutput=False)
    c_d = nc.declare_dram_parameter("consts", [128, C_TOTAL], FP, isOutput=False)
    w_d = nc.declare_dram_parameter("wmat", [128, W_TOTAL], FR, isOutput=False)
    o_d = nc.declare_dram_parameter("out", [R + PAD_ROWS, 16], FP, isOutput=True)

    with tile.TileContext(nc) as tc:
        with (
            tc.tile_pool(name="consts", bufs=1) as cp,
            tc.tile_pool(name="nat", bufs=NAT_BUFS) as natp,
            tc.tile_pool(name="sb", bufs=2) as sbp,
            tc.tile_pool(name="pshp", bufs=4, space="PSUM") as pshp,
        ):
            consts = cp.tile([128, C_TOTAL], FP)
            nc.sync.dma_start(out=consts, in_=c_d[:, :])
            wmat = cp.tile([128, W_TOTAL], FR)
            nc.sync.dma_start(out=wmat, in_=w_d[:, :])
            lhsT = {}
            for k in range(5):
                for bi, beta in ((0, "s"), (1, "b")):
                    blk = k * 2 + bi
                    lhsT[(k, beta)] = wmat[:, blk * 128:(blk + 1) * 128]
            bias = {}
            for k in range(4):
                bias[(k, "s")] = consts[:, C_BIAS + k:C_BIAS + k + 1]
                bias[(k, "b")] = consts[:, C_BIAS + 4 + k:C_BIAS + 5 + k]
            bias_e = consts[:, C_BIAS + 8:C_BIAS + 9]
            bias_b5 = consts[:, C_BIAS + 9:C_BIAS + 10]

            # warm up engines
            wu_ps = pshp.tile([128, 1024], FP, tag="hp")
            nc.tensor.matmul(wu_ps[:, 0:128], lhsT[(0, "s")],
                             lhsT[(0, "s")], start=True, stop=True)
            wu1 = sbp.tile([128, 1], FP, tag="wu", bufs=2)
            nc.scalar.copy(out=wu1, in_=bias_e)
            wu2 = sbp.tile([128, 1], FP, tag="wu")
            nc.vector.tensor_copy(out=wu2, in_=bias_e)
            wu3 = sbp.tile([128, 1], FP, tag="wu")
            nc.gpsimd.tensor_copy(out=wu3, in_=bias_e)

            nats = {}
            x0s = {}
            zrts = {}
            curs = {}
            ebs = {}
            hp5s = {}
            tmps = {}
            yrfs = {}
            tail_dmas = []

            def dma_in(m):
                nat = natp.tile([128, 2048], FP, tag="nat")
                nats[m] = nat
                nc.sync.dma_start(
                    out=nat.rearrange("p (s g f) -> p s g f", s=2, g=64, f=16),
                    in_=z_d[m * ROWS_PER_MACRO:(m + 1) * ROWS_PER_MACRO, :]
                    .rearrange("(s p g) f -> p s g f", s=2, p=128, g=64),
                )

            def head_st_zl(m):
                nat = nats[m]
                x0st = sbp.tile([128, 1024], FP, tag="x0st", bufs=2)
                nc.vector.transpose(
                    out=_ap(x0st, 0, [[1, 1024]]),
                    in_=_ap(nat, 0, ST_DIMS),
                )
                # fp32r rounding copy (DVE 2x dual-port: ~0.7us)
                x0 = sbp.tile([128, 1024], FR, tag="x0", bufs=3)
                nc.vector.tensor_copy(out=x0, in_=x0st)
                x0s[m] = x0
                curs[m] = {"s": x0, "b": x0}

            def head_st_zr(m):
                zrt = sbp.tile([128, 1024], FP, tag="zrt", bufs=5)
                nc.vector.transpose(
                    out=_ap(zrt, 0, [[1, 1024]]),
                    in_=_ap(nats[m], 8, ST_DIMS),
                )
                zrts[m] = zrt

            def leaky(k, beta, hp):
                hout = sbp.tile([128, 1024], FR, tag="h", bufs=12)
                eng = LEAKY_ENGINE[(k, beta)]
                if eng == "split":
                    c0, c1 = slice(0, 512), slice(512, 1024)
                    nc.vector._custom_dve(
                        leaky_op, out=hout[:, c0], in0=hp[:, c0],
                        s0=bias[(k, beta)], s1=0.0, imm2=ALPHA,
                    )
                    nc.scalar.activation(
                        out=hout[:, c1], in_=hp[:, c1],
                        func=mybir.ActivationFunctionType.Prelu,
                        bias=bias[(k, beta)], scale=1.0, alpha=ALPHA,
                    )
                elif eng == "act":
                    nc.scalar.activation(
                        out=hout, in_=hp,
                        func=mybir.ActivationFunctionType.Prelu,
                        bias=bias[(k, beta)], scale=1.0, alpha=ALPHA,
                    )
                else:
                    nc.vector._custom_dve(
                        leaky_op, out=hout, in0=hp,
                        s0=bias[(k, beta)], s1=0.0, imm2=ALPHA,
                    )
                return hout

            hps2 = {}

            def body_mm(m, k):
                cur = curs[m]
                hps = {}
                for beta in ("s", "b"):
                    hp = pshp.tile([128, 1024], FP, tag="hp")
                    for c in range(2):
                        nc.tensor.matmul(
                            hp[:, c * 512:(c + 1) * 512],
                            lhsT[(k, beta)],
                            cur[beta][:, c * 512:(c + 1) * 512],
                            start=True, stop=True,
                        )
                    hps[beta] = hp
                hps2[(m, k)] = hps

            def body_act(m, k):
                cur = curs[m]
                hps = hps2.pop((m, k))
                for beta in ("s", "b"):
                    cur[beta] = leaky(k, beta, hps[beta])

            def body_layer(m, k):
                body_mm(m, k)
                body_act(m, k)

            def body_l5_mm(m, beta):
                cur = curs[m]
                hp = pshp.tile([128, 1024], FP, tag="hp")
                for c in range(2):
                    nc.tensor.matmul(
                        hp[:, c * 512:(c + 1) * 512],
                        lhsT[(4, beta)],
                        cur[beta][:, c * 512:(c + 1) * 512],
                        start=True, stop=True,
                    )
                hp5s.setdefault(m, {})[beta] = hp
                if beta == "b":
                    curs.pop(m)
                    x0s.pop(m)

            def exp_b(m):
                # e = exp(hp5s + b5s) -> bf16 (ACT); emitted at iter start so
                # ACT has ready work at the boundary
                hp5 = hp5s[m]
                ebf_e = sbp.tile([128, 1024], FP, tag="eb", bufs=3)
                nc.scalar.activation(
                    out=ebf_e, in_=hp5["s"],
                    func=mybir.ActivationFunctionType.Exp,
                    bias=bias_e, scale=1.0,
                )
                ebs[m] = ebf_e

            def tail_tmp(m):
                # tmp = e * zr_fm on Pool (bf16 in, fp32 out for precision)
                tmp = sbp.tile([128, 1024], FP, tag="tmp", bufs=2)
                nc.gpsimd.tensor_mul(out=tmp, in0=ebs.pop(m), in1=zrts.pop(m))
                tmps[m] = tmp

            def tail_stt(m):
                # yr_fm = (hp5b + bias_b5) + tmp   (one DVE STT, fp32)
                hp5 = hp5s.pop(m)
                yrf = sbp.tile([128, 1024], FP, tag="yrf", bufs=2)
                nc.vector.scalar_tensor_tensor(
                    out=yrf, in0=hp5["b"], scalar=bias_b5, in1=tmps.pop(m),
                    op0=mybir.AluOpType.add, op1=mybir.AluOpType.add,
                )
                yrfs[m] = yrf

            def tail_styr(m):
                # yr feature-major -> nat zr columns (in-place, fp32 ST)
                nat = nats[m]
                nc.vector.transpose(
                    out=_ap(nat, 8, ST_DIMS),
                    in_=_ap(yrfs.pop(m), 0, [[1, 1024]]),
                )

            def tail_out(m):
                nat = nats.pop(m)
                out_dma = nc.sync.dma_start(
                    out=o_d[m * ROWS_PER_MACRO:(m + 1) * ROWS_PER_MACRO, :]
                    .rearrange("(s p g) f -> p s g f", s=2, p=128, g=64),
                    in_=nat.rearrange("p (s g f) -> p s g f", s=2, g=64, f=16),
                )
                if m >= MACROS - NAT_BUFS:
                    tail_dmas.append(out_dma)

            # ---- software-pipelined emission (4 deep, body split across two
            # iterations):  iter: head-cast(it) | exp/tail(it-3) |
            #               L2-3+L5(it-2) | L0-1(it-1) | head-st(it)
            for m in range(5):
                dma_in(m)
            for it in range(MACROS + 3):
                blA = it - 1         # early body (layers 0-1)
                blB = it - 2         # late body (layers 2-3, L5)
                tl = it - 3          # tail macro
                has_a = 0 <= blA < MACROS
                has_b = 0 <= blB < MACROS
                has_t = 0 <= tl < MACROS
                if it < MACROS:
                    head_st_zl(it)
                if has_t:
                    tail_tmp(tl)
                if has_b:
                    body_layer(blB, 2)
                if it < MACROS:
                    head_st_zr(it)
                if has_t:
                    tail_stt(tl)
                if has_a:
                    body_layer(blA, 0)
                if has_b:
                    body_layer(blB, 3)
                if has_a:
                    body_layer(blA, 1)
                if has_t:
                    tail_styr(tl)
                if has_b:
                    body_l5_mm(blB, "s")
                if has_b:
                    exp_b(blB)
                if has_b:
                    body_l5_mm(blB, "b")
                if it + 5 < MACROS:
                    dma_in(it + 5)
                if has_t:
                    tail_out(tl)

            flush = sbp.tile([128, 1], FP, tag="wu")
            fl = nc.vector.tensor_copy(out=flush, in_=bias_e)
            for dma in tail_dmas:
                _add_dep_helper(fl.ins, dma.ins, sync=True,
                                reason="drain tail out-DMAs before kernel end")

    nc.finalize()
    return nc


_NC_CACHE = None


def kernel(z, ws_logs, bs_logs, ws_b, bs_b):
    global _NC_CACHE, LAST_RESULTS
    z = np.asarray(z, np.float32)
    assert z.shape == (BATCH, 16)
    consts, wmat_bf = _build_consts(ws_logs, bs_logs, ws_b, bs_b)

    if _NC_CACHE is None:
        _NC_CACHE = _build_nc()
    nc = _NC_CACHE

    in_maps = []
    for s in STARTS:
        zp = np.zeros((R + PAD_ROWS, 16), np.float32)
        zp[:R] = z[s:s + R]
        in_maps.append({"z": zp, "consts": consts, "wmat": wmat_bf})
    trace = bool(os.environ.get("AFFINE_TRACE"))
    res = run_bass_kernel_spmd(nc, in_maps, core_ids=list(range(N_CORES)), trace=trace)
    LAST_RESULTS = res

    out = np.empty((BATCH, 16), np.float32)
    for c in range(N_CORES):
        out[STARTS[c]:STARTS[c] + R] = res.results[c]["out"][:R]
    return out
